# revision 3
# baseline (speedup 1.0000x reference)
"""Trainium2 Bass kernel for nn_Decoder8to4 — v5: v2 + fp8 DoubleRow.

On top of v2's folds (o folded into r/z weights; G = Wih_z@z and
h0 = tanh(Wi z + bi) computed on host):

  * The three h-contraction matmul groups (r, z via W' = Whh_rz + Wih_o,rz@Wo;
    n via Whh_n) run in fp8e4m3 with perf_mode=DoubleRow: K=256 per
    instruction, so 4 matmuls instead of 8 per gate per H-tile.
  * Scaling: weights x512, h x16 -> PSUM carries 8192x the true value.
    G is uploaded x8192, ob (o feedback) is staged x8192, bhh_n bias col
    x8192; the sigmoid/tanh activations apply scale=1/8192.
  * h state kept twice: bf16 tiles (feed the bf16 Wo output matmuls) and an
    fp8 [128, 8, 512] DoubleRow-layout tile per stream (x16), written by the
    scalar engine.

Per (k, s) per step: r: 1+4, z: 1+4, a: 4, b: 2 = 16 matmuls (33 in the
original baseline), plus 8 bf16 Wo matmuls per stream -> 272/step vs 544.
"""

import numpy as np
import ml_dtypes

import concourse.bacc as bacc
import concourse.mybir as mybir
import concourse.tile as tile
from concourse.bass_utils import run_bass_kernel_spmd

BF16 = ml_dtypes.bfloat16
F8 = ml_dtypes.float8_e4m3

B = 4096
HID = 1024
ZDIM = 256
ODIM = 128
T = 32
N_CORES = 8
BLOC = B // 4
P = 128
KH = HID // P
KD = KH // 2           # 4 DoubleRow K-steps
NS = 2
SB = BLOC // NS
TSTAGE = 2

SW = 512.0             # fp8 weight scale
SH = 16.0              # fp8 h scale
SC = SW * SH           # PSUM scale (8192)

F32 = mybir.dt.float32
BF = mybir.dt.bfloat16
F8D = mybir.dt.float8e4
AF = mybir.ActivationFunctionType
ALU = mybir.AluOpType
PM = mybir.MatmulPerfMode

# bias columns in packed [128, 58] tensor
_BRZ0 = 0      # 16: r/z bias at t=0 (incl. SOS)
_BRZ = 16      # 16: r/z bias t>=1 (incl. Wih_o,rz @ bo fold)
_BHN = 32      # 8: bhh n-part, x8192
_BIN0 = 40     # 8: bih n-part at t=0 (incl. SOS)
_BIN = 48      # 8: bih n-part
_BO = 56       # 1: output bias
_BOS = 57      # 1: output bias x8192


def build_program(loop_reps=None, dma_mode="sync"):
    nc = bacc.Bacc("TRN2", target_bir_lowering=False, debug=False)

    w8rz_d = nc.declare_dram_parameter("w8rz", [P, KH, 2 * HID], F8D, isOutput=False)
    w8n_d = nc.declare_dram_parameter("w8n", [P, KH, HID], F8D, isOutput=False)
    wio = nc.declare_dram_parameter("wio", [ODIM, 3 * HID], BF, isOutput=False)
    wot_d = nc.declare_dram_parameter("wot", [HID, ODIM], BF, isOutput=False)
    g_d = nc.declare_dram_parameter("g", [3 * HID, BLOC], BF, isOutput=False)
    h0_d = nc.declare_dram_parameter("h0", [HID, BLOC], BF, isOutput=False)
    h8_d = nc.declare_dram_parameter("h8", [P, KH, BLOC], F8D, isOutput=False)
    id_d = nc.declare_dram_parameter("id", [P, P], BF, isOutput=False)
    biases = nc.declare_dram_parameter("biases", [P, 58], F32, isOutput=False)
    out = nc.declare_dram_parameter(
        "out", [T // TSTAGE, ODIM, TSTAGE * BLOC], F32, isOutput=True
    )

    with tile.TileContext(nc) as tc:
        import contextlib

        with contextlib.ExitStack() as ctx:
            wpool = ctx.enter_context(tc.tile_pool(name="w", bufs=1))
            dbuf = ctx.enter_context(tc.tile_pool(name="dbuf", bufs=2))
            tmp = ctx.enter_context(tc.tile_pool(name="tmp", bufs=2))
            psum = ctx.enter_context(tc.tile_pool(name="ps", bufs=1, space="PSUM"))

            w8rz = wpool.tile([P, KH, 2 * HID], F8D, tag="w8rz", name="w8rz")
            nc.sync.dma_start(w8rz[:], w8rz_d[:, :, :])
            w8n = wpool.tile([P, KH, HID], F8D, tag="w8n", name="w8n")
            nc.sync.dma_start(w8n[:], w8n_d[:, :, :])
            wo_t = wpool.tile([P, 3 * HID], BF, tag="wio", name="wio")
            nc.sync.dma_start(wo_t[:], wio[:, :])
            wot = []
            for j in range(KH):
                t_ = wpool.tile([P, ODIM], BF, tag=f"wot{j}", name=f"wot{j}")
                nc.sync.dma_start(t_[:], wot_d[j * P : (j + 1) * P, :])
                wot.append(t_)
            gt = []
            for m in range(3 * KH):
                t_ = wpool.tile([P, BLOC], BF, tag=f"g{m}", name=f"g{m}")
                nc.sync.dma_start(t_[:], g_d[m * P : (m + 1) * P, :])
                gt.append(t_)
            idt = wpool.tile([P, P], BF, tag="id", name="id")
            nc.sync.dma_start(idt[:], id_d[:, :])
            bias = wpool.tile([P, 58], F32, tag="bias", name="bias")
            nc.sync.dma_start(bias[:], biases[:])

            def bcol(c):
                return bias[:, c : c + 1]

            loop_cm = (
                tc.For_i(0, loop_reps, 1) if loop_reps else contextlib.nullcontext()
            )
            ctx.enter_context(loop_cm)

            hb = [[None] * KH for _ in range(NS)]
            h8 = [None] * NS
            ob = [None] * NS
            stage = None

            def ssl(s):
                return slice(s * SB, (s + 1) * SB)

            # initial state from host; oneg = -(Wo @ h0) * SC
            for s in range(NS):
                for k in range(KH):
                    hb[s][k] = dbuf.tile([P, SB], BF, tag=f"hb{s}_{k}", name=f"hb{s}_{k}")
                    nc.sync.dma_start(hb[s][k][:], h0_d[k * P : (k + 1) * P, ssl(s)])
                h8[s] = dbuf.tile([P, KH, SB], F8D, tag=f"h8{s}", name=f"h8{s}")
                nc.sync.dma_start(h8[s][:], h8_d[:, :, ssl(s)])
            for s in range(NS):
                po = psum.tile([P, SB], F32, tag=f"pz{s}", name=f"poneg{s}")
                for j in range(KH):
                    nc.tensor.matmul(
                        po[:], wot[j][:], hb[s][j][:],
                        start=(j == 0), stop=(j == KH - 1),
                    )
                ob[s] = dbuf.tile([P, SB], BF, tag=f"ob{s}", name=f"ob{s}")
                nc.scalar.activation(ob[s][:], po[:], AF.Identity, scale=-SC)

            def emit_A(t, s, k, hb_cur, h8_cur):
                first = t == 0
                brz = _BRZ0 if first else _BRZ

                pg = {}
                for gate, m in (("r", k), ("z", KH + k)):
                    p_ = psum.tile([P, SB], F32, tag=f"p{gate}{s}", name=f"p{gate}{s}")
                    for j in range(KD):
                        nc.tensor.matmul(
                            p_[:],
                            w8rz[:, 2 * j : 2 * j + 2, m * P : (m + 1) * P],
                            h8_cur[s][:, 2 * j : 2 * j + 2, :],
                            start=(j == 0),
                            stop=(j == KD - 1 and not first),
                            perf_mode=PM.DoubleRow,
                        )
                    if first:  # step-0 correction: + Wih_o,rz @ oneg
                        nc.tensor.matmul(
                            p_[:],
                            wo_t[:, m * P : (m + 1) * P],
                            ob[s][:],
                            start=False,
                            stop=True,
                        )
                    pg[gate] = p_
                # G_r / G_z injected on DVE instead of PE identity matmuls
                ur = tmp.tile([P, SB], F32, tag=f"ur{s}", name=f"ur{s}")
                uz = tmp.tile([P, SB], F32, tag=f"uz{s}", name=f"uz{s}")
                nc.vector.tensor_add(ur[:], pg["r"][:], gt[k][:, ssl(s)])
                nc.vector.tensor_add(uz[:], pg["z"][:], gt[KH + k][:, ssl(s)])
                pg = {"r": ur, "z": uz}
                m = 2 * KH + k
                pa = psum.tile([P, SB], F32, tag=f"pa{s}", name=f"pa{s}")
                for j in range(KD):
                    nc.tensor.matmul(
                        pa[:],
                        w8n[:, 2 * j : 2 * j + 2, k * P : (k + 1) * P],
                        h8_cur[s][:, 2 * j : 2 * j + 2, :],
                        start=(j == 0),
                        stop=(j == KD - 1),
                        perf_mode=PM.DoubleRow,
                    )
                pb = None
                if not first:  # Wih_o,n @ (o_{t-1} * SC); G_n added on DVE
                    pb = psum.tile([P, SB], F32, tag=f"pb{s}", name=f"pb{s}")
                    nc.tensor.matmul(
                        pb[:], wo_t[:, m * P : (m + 1) * P], ob[s][:],
                        start=True, stop=True,
                    )
                rt = tmp.tile([P, SB], BF, tag=f"rt{s}", name=f"rt{s}")
                zt = tmp.tile([P, SB], BF, tag=f"zt{s}", name=f"zt{s}")
                nc.scalar.activation(
                    rt[:], pg["r"][:], AF.Sigmoid, bias=bcol(brz + k), scale=1.0 / SC
                )
                nc.scalar.activation(
                    zt[:], pg["z"][:], AF.Sigmoid, bias=bcol(brz + KH + k),
                    scale=1.0 / SC,
                )
                t1 = tmp.tile([P, SB], F32, tag=f"t1{s}", name=f"t1{s}")
                nc.vector.scalar_tensor_tensor(
                    t1[:], pa[:], bcol(_BHN + k), rt[:], op0=ALU.add, op1=ALU.mult
                )
                if pb is not None:
                    nc.vector.tensor_add(t1[:], t1[:], pb[:])
                nc.vector.tensor_add(t1[:], t1[:], gt[m][:, ssl(s)])
                return zt, t1

            def emit_B(t, s, k, zt, t1, hb_old, h8_cur):
                bin_ = _BIN0 if t == 0 else _BIN
                nt = tmp.tile([P, SB], BF, tag=f"nt{s}", name=f"nt{s}")
                nc.scalar.activation(
                    nt[:], t1[:], AF.Tanh, bias=bcol(bin_ + k), scale=1.0 / SC
                )
                dt_ = tmp.tile([P, SB], BF, tag=f"dt{s}", name=f"dt{s}")
                nc.vector.scalar_tensor_tensor(
                    dt_[:], nt[:], -1.0, hb_old[:], op0=ALU.mult, op1=ALU.add
                )
                nc.vector.tensor_mul(dt_[:], zt[:], dt_[:])
                hnew = dbuf.tile([P, SB], BF, tag=f"hb{s}_{k}", name=f"hb{s}_{k}")
                nc.vector.tensor_add(hnew[:], nt[:], dt_[:])
                nc.scalar.activation(
                    h8_cur[s][:, k, :], hnew[:], AF.Identity, scale=SH
                )
                return hnew

            for t in range(T):
                hb_old = [list(hb[s]) for s in range(NS)]
                h8_old = list(h8)
                hb_new = [[None] * KH for _ in range(NS)]
                h8_new = [
                    dbuf.tile([P, KH, SB], F8D, tag=f"h8{s}", name=f"h8{s}")
                    for s in range(NS)
                ]
                pend = [None] * NS
                for k in range(KH + 1):
                    for s in range(NS):
                        if k < KH:
                            zt, t1 = emit_A(t, s, k, hb_old, h8_old)
                            nxt = (k, zt, t1)
                        else:
                            nxt = None
                        if pend[s] is not None:
                            pk, pzt, pt1 = pend[s]
                            hb_new[s][pk] = emit_B(
                                t, s, pk, pzt, pt1, hb_old[s][pk], h8_new
                            )
                        pend[s] = nxt
                hb = hb_new
                h8 = h8_new

                if t % TSTAGE == 0:
                    stage = tmp.tile(
                        [P, TSTAGE * BLOC], F32, tag="stage", name="stage", bufs=2,
                    )
                so = (t % TSTAGE) * BLOC
                for s in range(NS):
                    po = psum.tile([P, SB], F32, tag=f"pz{s}", name=f"po{s}")
                    for j in range(KH):
                        nc.tensor.matmul(
                            po[:], wot[j][:], hb[s][j][:],
                            start=(j == 0), stop=(j == KH - 1),
                        )
                    if t < T - 1:
                        ob[s] = dbuf.tile([P, SB], BF, tag=f"ob{s}", name=f"ob{s}")
                        nc.scalar.activation(
                            ob[s][:], po[:], AF.Identity, bias=bcol(_BOS), scale=SC
                        )
                    nc.scalar.activation(
                        stage[:, so + s * SB : so + (s + 1) * SB],
                        po[:],
                        AF.Identity,
                        bias=bcol(_BO),
                    )
                if t % TSTAGE == TSTAGE - 1 and dma_mode != "none":
                    nc.sync.dma_start(out[t // TSTAGE, :, :], stage[:])

    nc.compile()
    return nc


def prep_core_inputs(inputs, core, _cache={}):
    d, q = divmod(core, 4)
    sfx = str(d)
    z = np.asarray(inputs["z_8p" if d == 0 else "z_8r"], np.float32)
    if d not in _cache:
        Wi = np.asarray(inputs["Wi" + sfx], np.float32)
        bi = np.asarray(inputs["bi" + sfx], np.float32)
        Wih = np.asarray(inputs["Wih" + sfx], np.float32)
        Whh = np.asarray(inputs["Whh" + sfx], np.float32)
        bih = np.asarray(inputs["bih" + sfx], np.float32)
        bhh = np.asarray(inputs["bhh" + sfx], np.float32)
        Wo = np.asarray(inputs["Wo" + sfx], np.float32)
        bo = np.asarray(inputs["bo" + sfx], np.float32)

        H2 = 2 * HID
        Wf_rz = Whh[:H2] + Wih[:H2, :ODIM] @ Wo   # [2H, HID]
        # DoubleRow layout [P, KH, M]: (p, j, m) = W.T[j*P + p, m]
        w8rz = (
            np.ascontiguousarray(
                (Wf_rz.T * SW).reshape(KH, P, H2).transpose(1, 0, 2)
            ).astype(F8)
        )
        w8n = (
            np.ascontiguousarray(
                (Whh[H2:].T * SW).reshape(KH, P, HID).transpose(1, 0, 2)
            ).astype(F8)
        )
        sos = Wih[:, ODIM - 1]
        brzsum = bih[:H2] + bhh[:H2]
        obias = Wih[:H2, :ODIM] @ bo
        cols = [
            (brzsum + sos[:H2]).reshape(16, P).T,      # _BRZ0
            (brzsum + obias).reshape(16, P).T,         # _BRZ
            (bhh[H2:] * SC).reshape(KH, P).T,          # _BHN (x8192)
            (bih[H2:] + sos[H2:]).reshape(KH, P).T,    # _BIN0
            bih[H2:].reshape(KH, P).T,                 # _BIN
            bo.reshape(1, P).T,                        # _BO
            (bo * SC).reshape(1, P).T,                 # _BOS
        ]
        _cache[d] = {
            "w8rz": w8rz, "w8n": w8n,
            "wio": np.ascontiguousarray(Wih[:, :ODIM].T).astype(BF16),
            "wot": np.ascontiguousarray(Wo.T).astype(BF16),
            "id": np.eye(P, dtype=np.float32).astype(BF16),
            "biases": np.ascontiguousarray(np.concatenate(cols, axis=1), np.float32),
            "_Wihz": Wih[:, ODIM:],
            "_Wi": Wi, "_bi": bi,
        }
    c = _cache[d]
    zq = z[q * BLOC : (q + 1) * BLOC]  # [BLOC, ZDIM]
    g = ((c["_Wihz"] @ zq.T) * SC).astype(BF16)       # [3H, BLOC] x8192
    h0 = np.tanh(zq @ c["_Wi"].T + c["_bi"]).T        # [HID, BLOC]
    h8 = np.ascontiguousarray(
        (h0 * SH).reshape(KH, P, BLOC).transpose(1, 0, 2)
    ).astype(F8)
    return {
        "w8rz": c["w8rz"], "w8n": c["w8n"], "wio": c["wio"], "wot": c["wot"],
        "id": c["id"], "biases": c["biases"],
        "g": np.ascontiguousarray(g),
        "h0": np.ascontiguousarray(h0.astype(BF16)),
        "h8": h8,
    }


_NC_CACHE = None


def get_program():
    global _NC_CACHE
    if _NC_CACHE is None:
        _NC_CACHE = build_program()
    return _NC_CACHE


def run(inputs, **run_kwargs):
    nc = get_program()
    in_maps = [prep_core_inputs(inputs, c) for c in range(N_CORES)]
    res = run_bass_kernel_spmd(nc, in_maps, list(range(N_CORES)), **run_kwargs)
    outs = []
    for d in range(2):
        parts = []
        for q in range(4):
            o = res.results[d * 4 + q]["out"]  # [T/TS, ODIM, TS*BLOC]
            o = (
                o.reshape(T // TSTAGE, ODIM, TSTAGE, BLOC)
                .transpose(0, 2, 3, 1)
                .reshape(T, BLOC, ODIM)
                .transpose(1, 0, 2)
            )
            parts.append(np.ascontiguousarray(o))
        outs.append(np.concatenate(parts, axis=0))
    return (outs[0], outs[1]), res


def kernel(**inputs):
    (z4p, z4r), _ = run(inputs)
    return z4p, z4r
